# revision 1
# baseline (speedup 1.0000x reference)
"""3-layer GCN (message passing) on 8 Trainium2 NeuronCores.

Strategy (graph/data parallel, per sharding hint):
  - Nodes sharded by destination across 8 cores (6250 dst rows each);
    edges bucketed by dst owner on the host; weights replicated.
  - Per layer:  out = Ahat @ (z @ W^T) + b  ==  (Ahat @ z) @ W^T + b
    where Ahat = D^-1/2 (A+I) D^-1/2.  Each core computes its dst shard:
      1. gather z[src] rows (fp16) for its edges via dma_gather from a
         full local fp16 replica of z,
      2. scatter-add into 256-dst PSUM groups via one-hot matmul
         (one-hot built on DVE: (iota == dst_local) * norm),
      3. dense W^T matmul (feature-major), bias+ReLU on ACT,
      4. transpose to node-major and store the shard,
      5. AllGather the fp16 shards -> full z for the next layer.
  - Graph prep (degrees, norms, edge bucketing/padding) is host-side.
"""
import logging
import math
import re

import numpy as np

import concourse.bass as bass
import concourse.tile as tile
from concourse import bacc, mybir

N = 50000
E = 600000
D = 128
N_CORES = 8
SHARD = N // N_CORES          # 6250
GW = 256                      # dst-group width (psum group)
N_GROUPS = math.ceil(SHARD / GW)   # 25 (24*256 + 106)
HALF = N // 2                 # gather-table halves (int16 index limit)
IDX_PER_CALL = 1024
BLK = 128
F16 = mybir.dt.float16
F32 = mybir.dt.float32
I16 = mybir.dt.int16


# ---------------------------------------------------------------- host prep

def _wrap_idx(flat):
    """dma_gather index layout: [128, S/16] int16, idx i at [i%16, i//16],
    replicated across the 8 gpsimd 16-partition groups."""
    S = flat.shape[0]
    arr = np.zeros((128, S // 16), np.int16)
    w = flat.reshape(S // 16, 16).T          # [16, S/16]
    for grp in range(8):
        arr[grp * 16:(grp + 1) * 16, :] = w
    return arr


def prep_graph(edge_index):
    src = np.concatenate([edge_index[0].astype(np.int64), np.arange(N, dtype=np.int64)])
    dst = np.concatenate([edge_index[1].astype(np.int64), np.arange(N, dtype=np.int64)])
    deg = np.bincount(dst, minlength=N).astype(np.float64)
    dinv = 1.0 / np.sqrt(deg)
    norm = (dinv[src] * dinv[dst]).astype(np.float32)

    core = dst // SHARD
    gloc = (dst % SHARD) // GW
    half = (src >= HALF).astype(np.int64)
    cell = (core * N_GROUPS + gloc) * 2 + half

    counts = np.bincount(cell, minlength=N_CORES * N_GROUPS * 2)
    counts = counts.reshape(N_CORES, N_GROUPS, 2)
    B = np.ceil(counts / BLK).astype(np.int64).max(axis=0)   # [N_GROUPS, 2]

    # per-half streams; cell (g,h) occupies B[g,h]*BLK slots of stream h
    stream_blocks = [B[:, h].sum() for h in (0, 1)]
    ncalls = [math.ceil(sb * BLK / IDX_PER_CALL) for sb in stream_blocks]
    stream_slots = [nc_ * IDX_PER_CALL for nc_ in ncalls]
    cell_base = np.zeros((N_GROUPS, 2), np.int64)           # slot base within stream h
    for h in (0, 1):
        cell_base[:, h] = np.cumsum(B[:, h] * BLK) - B[:, h] * BLK

    # rank of each edge within its cell
    order = np.argsort(cell, kind="stable")
    cell_sorted = cell[order]
    starts = np.searchsorted(cell_sorted, np.arange(N_CORES * N_GROUPS * 2))
    rank = np.arange(cell.shape[0]) - starts[cell_sorted]
    # slot within the edge's (core, stream-h): cell_base + rank
    g_s = gloc[order]
    h_s = half[order]
    c_s = core[order]
    slot = cell_base[g_s, h_s] + rank

    idx16 = (src[order] - h_s * HALF).astype(np.int16)
    dstloc = ((dst[order] % SHARD) % GW).astype(np.float32)
    normv = norm[order].astype(np.float32)

    per_core = []
    NBs = [sl // BLK for sl in stream_slots]
    for c in range(N_CORES):
        m = c_s == c
        data = {}
        for h in (0, 1):
            mh = m & (h_s == h)
            idx_flat = np.zeros(stream_slots[h], np.int16)
            dl_flat = np.zeros(stream_slots[h], np.float32)
            nm_flat = np.zeros(stream_slots[h], np.float32)
            s = slot[mh]
            idx_flat[s] = idx16[mh]
            dl_flat[s] = dstloc[mh]
            nm_flat[s] = normv[mh]
            data[f"idx{h}"] = _wrap_idx(idx_flat)
            data[f"dl{h}"] = dl_flat.reshape(NBs[h], BLK).T.copy()   # [128, NB_h]
            data[f"nm{h}"] = nm_flat.reshape(NBs[h], BLK).T.copy()
        per_core.append(data)
    return B, ncalls, NBs, per_core


# ---------------------------------------------------------------- bass kernel

def build_nc(B, ncalls, NBs, ablate=(), reps=1, nq=1):
    """ablate: subset of {"gather", "onehot", "matmul", "collective", "dense"}
    — drop that phase (wrong results, used for perf bisection only).
    reps: repeat the whole 3-layer pipeline (for slope-based HW timing).
    nq: number of SWDGE queues to spread dma_gather calls across."""
    nc = bacc.Bacc("TRN2", target_bir_lowering=False, debug=False,
                   num_devices=N_CORES, num_swdge_queues=nq)

    x_tab = nc.dram_tensor("x_tab", [N, D], F16, kind="ExternalInput")
    idx_in = [nc.dram_tensor(f"idx{h}", [128, ncalls[h] * IDX_PER_CALL // 16], I16,
                             kind="ExternalInput") for h in (0, 1)]
    dl_in = [nc.dram_tensor(f"dl{h}", [128, NBs[h]], F32, kind="ExternalInput")
             for h in (0, 1)]
    nm_in = [nc.dram_tensor(f"nm{h}", [128, NBs[h]], F32, kind="ExternalInput")
             for h in (0, 1)]
    iota_in = nc.dram_tensor("iota", [128, GW], F16, kind="ExternalInput")
    id16_in = nc.dram_tensor("id16", [128, 128], F16, kind="ExternalInput")
    id32_in = nc.dram_tensor("id32", [128, 128], F32, kind="ExternalInput")
    w_in = [nc.dram_tensor(f"w{l}t", [D, D], F16, kind="ExternalInput")
            for l in range(3)]
    b_in = [nc.dram_tensor(f"b{l}", [128, 1], F32, kind="ExternalInput")
            for l in range(3)]
    y_out = nc.dram_tensor("y", [SHARD, D], F32, kind="ExternalOutput")

    zshard = [nc.dram_tensor(f"z{l}s", [SHARD, D], F16) for l in range(2)]
    zfull = [nc.dram_tensor(f"z{l}f", [N, D], F16, addr_space="Shared")
             for l in range(2)]

    with tile.TileContext(nc) as tc:
        with tc.tile_pool(name="const", bufs=1) as cpool, \
             tc.tile_pool(name="glo", bufs=4) as glo_pool, \
             tc.tile_pool(name="ghi", bufs=4) as ghi_pool, \
             tc.tile_pool(name="s", bufs=4) as s_pool, \
             tc.tile_pool(name="a", bufs=2) as a_pool, \
             tc.tile_pool(name="z", bufs=2) as z_pool, \
             tc.tile_pool(name="t", bufs=3) as t_pool, \
             tc.tile_pool(name="psa", bufs=2, space="PSUM") as psa_pool, \
             tc.tile_pool(name="pso", bufs=2, space="PSUM") as pso_pool, \
             tc.tile_pool(name="pst", bufs=2, space="PSUM") as pst_pool:

            # ---- constants
            idx_t, dl_t, nm_t = [], [], []
            for h in (0, 1):
                it = cpool.tile([128, ncalls[h] * IDX_PER_CALL // 16], I16, tag=f"idx{h}")
                nc.sync.dma_start(out=it[:], in_=idx_in[h][:, :])
                idx_t.append(it)
                dt_ = cpool.tile([128, NBs[h]], F32, tag=f"dl{h}")
                nc.sync.dma_start(out=dt_[:], in_=dl_in[h][:, :])
                dl_t.append(dt_)
                nt = cpool.tile([128, NBs[h]], F32, tag=f"nm{h}")
                nc.sync.dma_start(out=nt[:], in_=nm_in[h][:, :])
                nm_t.append(nt)
            iota_t = cpool.tile([128, GW], F16, tag="iota")
            nc.sync.dma_start(out=iota_t[:], in_=iota_in[:, :])
            id16_t = cpool.tile([128, 128], F16, tag="id16")
            nc.sync.dma_start(out=id16_t[:], in_=id16_in[:, :])
            id32_t = cpool.tile([128, 128], F32, tag="id32")
            nc.sync.dma_start(out=id32_t[:], in_=id32_in[:, :])
            w_t, b_t = [], []
            for l in range(3):
                wt = cpool.tile([D, D], F16, tag=f"w{l}")
                nc.sync.dma_start(out=wt[:], in_=w_in[l][:, :])
                w_t.append(wt)
                bt = cpool.tile([128, 1], F32, tag=f"b{l}")
                nc.sync.dma_start(out=bt[:], in_=b_in[l][:, :])
                b_t.append(bt)

            s_zero = None
            if "onehot" in ablate:
                s_zero = cpool.tile([128, GW], F16, tag="szero")
                nc.vector.memset(s_zero[:], 0.0)

            # block -> stream slot base for each (g, h)
            cell_base_blk = np.zeros((N_GROUPS, 2), np.int64)
            for h in (0, 1):
                cell_base_blk[:, h] = np.cumsum(B[:, h]) - B[:, h]

            for rep_layer in range(3 * reps):
                layer = rep_layer % 3
                z_tab = [x_tab, zfull[0], zfull[1]][layer]
                tabs = [z_tab[0:HALF, :], z_tab[HALF:N, :]]
                g_tiles = [{}, {}]   # per stream: call -> tile
                pools = [glo_pool, ghi_pool]

                def get_block(h, blk_i):
                    call = (blk_i * BLK) // IDX_PER_CALL
                    j = blk_i - call * (IDX_PER_CALL // BLK)
                    if call not in g_tiles[h]:
                        gt = pools[h].tile([128, IDX_PER_CALL // BLK, D], F16,
                                           tag=f"g{h}")
                        if "gather" not in ablate:
                            nc.gpsimd.dma_gather(
                                out_ap=gt[:],
                                in_ap=tabs[h],
                                idxs_ap=idx_t[h][:, call * (IDX_PER_CALL // 16):
                                                 (call + 1) * (IDX_PER_CALL // 16)],
                                num_idxs=IDX_PER_CALL,
                                num_idxs_reg=IDX_PER_CALL,
                                elem_size=D,
                                queue_num=(h + 2 * call) % nq,
                            )
                        else:
                            nc.vector.memset(gt[:, 0, :], 0.25)
                        g_tiles[h][call] = gt
                    return g_tiles[h][call][:, j, :]

                for g in range(N_GROUPS):
                    gw_act = min(GW, SHARD - g * GW)
                    nblk = int(B[g, 0] + B[g, 1])
                    psA = psa_pool.tile([128, GW], F32, tag="psa")
                    bi = 0
                    for h in (0, 1):
                        for i in range(int(B[g, h])):
                            blk_i = int(cell_base_blk[g, h] + i)
                            gblk = get_block(h, blk_i)
                            if "onehot" not in ablate:
                                sT = s_pool.tile([128, GW], F16, tag="s")
                                nc.vector.tensor_scalar(
                                    out=sT[:], in0=iota_t[:],
                                    scalar1=dl_t[h][:, blk_i:blk_i + 1],
                                    scalar2=nm_t[h][:, blk_i:blk_i + 1],
                                    op0=mybir.AluOpType.is_equal,
                                    op1=mybir.AluOpType.mult,
                                )
                            else:
                                sT = s_zero
                            if "matmul" not in ablate:
                                nc.tensor.matmul(
                                    out=psA[:], lhsT=gblk, rhs=sT[:],
                                    start=(bi == 0), stop=(bi == nblk - 1),
                                )
                            elif bi == 0:
                                nc.tensor.matmul(out=psA[:], lhsT=gblk, rhs=sT[:],
                                                 start=True, stop=True)
                            bi += 1

                    aT = a_pool.tile([128, GW], F16, tag="a")
                    nc.vector.tensor_copy(out=aT[:], in_=psA[:])
                    psO = pso_pool.tile([128, GW], F32, tag="pso")
                    nc.tensor.matmul(out=psO[:], lhsT=w_t[layer][:], rhs=aT[:],
                                     start=True, stop=True)

                    if layer < 2:
                        zT = z_pool.tile([128, GW], F16, tag="z16")
                        nc.scalar.activation(out=zT[:], in_=psO[:],
                                             func=mybir.ActivationFunctionType.Relu,
                                             bias=b_t[layer][:])
                        ident = id16_t
                        odt = F16
                        dest = zshard[layer]
                    else:
                        zT = z_pool.tile([128, GW], F32, tag="z32")
                        nc.vector.tensor_scalar(
                            out=zT[:], in0=psO[:], scalar1=b_t[layer][:],
                            scalar2=None, op0=mybir.AluOpType.add)
                        ident = id32_t
                        odt = F32
                        dest = y_out

                    for t in range(math.ceil(gw_act / 128)):
                        width = min(128, gw_act - t * 128)
                        psT = pst_pool.tile([128, 128], odt, tag="pst")
                        nc.tensor.transpose(out=psT[:], in_=zT[:, t * 128:(t + 1) * 128],
                                            identity=ident[:])
                        ts_ = t_pool.tile([128, 128], odt, tag="t")
                        nc.vector.tensor_copy(out=ts_[:], in_=psT[:])
                        base = g * GW + t * 128
                        nc.sync.dma_start(out=dest[base:base + width, :],
                                          in_=ts_[:width, :])

                if layer < 2:
                    if "collective" not in ablate:
                        nc.gpsimd.collective_compute(
                            "AllGather", mybir.AluOpType.bypass,
                            replica_groups=[list(range(N_CORES))],
                            ins=[zshard[layer].ap().opt()],
                            outs=[zfull[layer].ap().opt()],
                        )
                    else:
                        for c in range(N_CORES):
                            nc.sync.dma_start(
                                out=zfull[layer][c * SHARD:(c + 1) * SHARD, :],
                                in_=zshard[layer][:, :])

    nc.compile()
    return nc


class _MakespanFilter(logging.Filter):
    """Captures the Tile scheduling sim's predicted makespan."""

    def __init__(self):
        super().__init__()
        self.times = []

    def filter(self, record):
        m = re.search(r"Simulation completed at time (\d+)", record.getMessage())
        if m:
            self.times.append(int(m.group(1)))
        return True


def build_with_makespan(*args, **kwargs):
    lg = logging.getLogger("concourse")
    old_level = lg.level
    f = _MakespanFilter()
    lg.addFilter(f)
    lg.setLevel(logging.DEBUG)
    try:
        nc = build_nc(*args, **kwargs)
    finally:
        lg.removeFilter(f)
        lg.setLevel(old_level)
    makespan = max(f.times) if f.times else None
    return nc, makespan


# ---------------------------------------------------------------- runner

class SpmdRunner:
    """Persistent jitted SPMD executor (axon/PJRT path, jit built once)."""

    def __init__(self, nc, n_cores):
        import jax
        from jax.sharding import Mesh, PartitionSpec
        from jax.experimental.shard_map import shard_map
        from concourse.bass2jax import (_bass_exec_p, install_neuronx_cc_hook,
                                        partition_id_tensor)
        install_neuronx_cc_hook()
        self.jax = jax
        self.nc = nc
        self.n_cores = n_cores
        partition_name = nc.partition_id_tensor.name if nc.partition_id_tensor else None
        in_names, out_names, out_avals, zero_outs = [], [], [], []
        for alloc in nc.m.functions[0].allocations:
            if not isinstance(alloc, mybir.MemoryLocationSet):
                continue
            name = alloc.memorylocations[0].name
            if alloc.kind == "ExternalInput":
                if name != partition_name:
                    in_names.append(name)
            elif alloc.kind == "ExternalOutput":
                shape = tuple(alloc.tensor_shape)
                dtype = mybir.dt.np(alloc.dtype)
                out_names.append(name)
                out_avals.append(jax.core.ShapedArray(shape, dtype))
                zero_outs.append(np.zeros(shape, dtype))
        self.in_names, self.out_names = in_names, out_names
        self.out_avals, self.zero_outs = out_avals, zero_outs
        n_params, n_outs = len(in_names), len(out_avals)
        all_in = list(in_names) + list(out_names)
        if partition_name is not None:
            all_in.append(partition_name)

        def _body(*args):
            operands = list(args)
            if partition_name is not None:
                operands.append(partition_id_tensor())
            outs = _bass_exec_p.bind(
                *operands, out_avals=tuple(out_avals), in_names=tuple(all_in),
                out_names=tuple(out_names), lowering_input_output_aliases=(),
                sim_require_finite=True, sim_require_nnan=True, nc=nc)
            return tuple(outs)

        devices = jax.devices()[:n_cores]
        mesh = Mesh(np.asarray(devices), ("core",))
        from jax.sharding import PartitionSpec as P
        self._fn = jax.jit(
            shard_map(_body, mesh=mesh,
                      in_specs=(P("core"),) * (n_params + n_outs),
                      out_specs=(P("core"),) * n_outs, check_rep=False),
            keep_unused=True)
        self._staged = None

    def stage_inputs(self, in_maps):
        n = self.n_cores
        concat = [np.concatenate([np.asarray(in_maps[c][nm]) for c in range(n)], axis=0)
                  for nm in self.in_names]
        concat += [np.zeros((n * z.shape[0], *z.shape[1:]), z.dtype)
                   for z in self.zero_outs]
        self._staged = [self.jax.device_put(a) for a in concat]

    def run(self):
        outs = self._fn(*self._staged)
        self.jax.block_until_ready(outs)
        return outs

    def results(self, outs):
        res = []
        for c in range(self.n_cores):
            m = {}
            for i, nm in enumerate(self.out_names):
                full = np.asarray(outs[i])
                m[nm] = full.reshape(self.n_cores, *self.out_avals[i].shape)[c]
            res.append(m)
        return res


_CACHE = {}


def _get_built(B_key, B, ncalls, NBs):
    if B_key not in _CACHE:
        nc, makespan = build_with_makespan(B, ncalls, NBs)
        if makespan:
            print(f"[kernel] predicted makespan: {makespan} ns")
        _CACHE[B_key] = (nc, SpmdRunner(nc, N_CORES))
    return _CACHE[B_key]


def kernel(x, edge_index, W1, b1, W2, b2, W3, b3):
    x = np.asarray(x)
    edge_index = np.asarray(edge_index)
    B, ncalls, NBs, per_core = prep_graph(edge_index)
    B_key = (tuple(B.flatten().tolist()), tuple(ncalls))
    nc, runner = _get_built(B_key, B, ncalls, NBs)

    x16 = x.astype(np.float16)
    iota = np.tile(np.arange(GW, dtype=np.float16), (128, 1))
    ident = np.eye(128)
    common = {
        "x_tab": x16,
        "iota": iota,
        "id16": ident.astype(np.float16),
        "id32": ident.astype(np.float32),
        "w0t": np.asarray(W1).T.astype(np.float16),
        "w1t": np.asarray(W2).T.astype(np.float16),
        "w2t": np.asarray(W3).T.astype(np.float16),
        "b0": np.asarray(b1).reshape(128, 1).astype(np.float32),
        "b1": np.asarray(b2).reshape(128, 1).astype(np.float32),
        "b2": np.asarray(b3).reshape(128, 1).astype(np.float32),
    }
    in_maps = [{**common, **per_core[c]} for c in range(N_CORES)]
    runner.stage_inputs(in_maps)
    outs = runner.run()
    res = runner.results(outs)
    return np.concatenate([res[c]["y"] for c in range(N_CORES)], axis=0)

